# revision 19
# baseline (speedup 1.0000x reference)
"""Trainium2 Bass kernel for the 4-layer sum/product circuit
(nn_KnowledgeLayer): h = enc(x); h = h[idx0].prod(1); h = h[idx1].sum(1);
h = h[idx2].prod(1); h = h[idx3].sum(1).

Strategy (2-way stream shard x 4-way batch shard over 8 cores):
  * Host composes the four index maps into ONE gather from a 4098-row
    table (enc = [x | 1-x | 0 | 1]) producing two leaf-operand streams in
    an interleaved order chosen so every later layer reduces ADJACENT
    free-dim blocks within a partition.
  * Core (g, b) handles stream half g (16384 of 32768 leaf pairs → 2048 of
    4096 output rows) for batch quarter b (256 of 1024 columns).  Halving
    the per-core descriptor count halves the SWDGE descriptor-generation
    time (the G=1 bottleneck) while doubling descriptor payload to 512 B
    (f16), keeping the DMA engines byte-efficient.
  * Device: build f16 enc table in DRAM, dma_gather (SWDGE) the two
    streams chunk by chunk into SBUF [128, G, 256] round-robin over 4
    SWDGE queues, then DVE does mul / add / mul / add over strided block
    pairs (h0/h1 in f16 for 2x DVE mode) and DMAs h3 out.

The bass program is identical for all 8 cores (pure SPMD); per-core data
(x batch slice + stream-half index streams) is supplied via in_maps.
"""

import numpy as np

N_VARS = 2048
BATCH = 1024
NCORES = 8
GSHARD = 2                        # stream shards
BSHARD = NCORES // GSHARD         # 4 batch shards
BSLICE = BATCH // BSHARD          # 256 batch columns per core
TABLE_ROWS = 2 * N_VARS + 2       # 4098
NIDX_FULL = 32768                 # total leaf pairs
NIDX = NIDX_FULL // GSHARD        # 16384 leaf pairs per core
NOUT_FULL = 4096
NOUT = NOUT_FULL // GSHARD        # 2048 h3 rows per core

# chunking: NIDX/1024 = 16 gather-call blocks; CBLK h3-row-blocks per chunk
CBLK = 1
NCHUNK = (NOUT // 128) // CBLK    # 16
CH_IDX = CBLK * 8 * 128           # gathered rows per stream per chunk = 1024

NQ = 4                            # SWDGE queues (ucode max)


# ----------------------------------------------------------------------------
# host-side index preparation
# ----------------------------------------------------------------------------

def _compose_indices(idx0, idx1, idx2, idx3):
    J = idx3.reshape(-1)
    K = idx2[J].reshape(-1)
    L = idx1[K].reshape(-1)
    AB = idx0[L]
    A, B = AB[:, 0].astype(np.int64), AB[:, 1].astype(np.int64)

    def remap(e):
        out = np.empty_like(e)
        out[e == 0] = 2 * N_VARS
        out[e == 1] = 2 * N_VARS + 1
        even = (e >= 2) & (e % 2 == 0)
        out[even] = (e[even] - 2) // 2
        odd = (e >= 3) & (e % 2 == 1)
        out[odd] = N_VARS + (e[odd] - 3) // 2
        return out

    return remap(A), remap(B)


def _shard_perm(stream_idx, g):
    # slice stream half g, then interleave so DVE reduces adjacent blocks
    sl = stream_idx[g * NIDX:(g + 1) * NIDX]
    s = np.arange(NIDX)
    q = 8 * ((s // 1024) * 128 + (s % 128)) + (s // 128) % 8
    return sl[q]


def _wrap_idx(stream_idx):
    w = stream_idx.reshape(-1, 16).T.astype(np.int16)   # [16, NIDX//16]
    return np.ascontiguousarray(np.tile(w, (8, 1)))     # [128, NIDX//16]


# ----------------------------------------------------------------------------
# bass program (built once, cached)
# ----------------------------------------------------------------------------

_CACHED = {}


def _build_program():
    import concourse.bacc as bacc
    import concourse.mybir as mybir
    from concourse import library_config
    from concourse.tile import TileContext

    f32 = mybir.dt.float32
    f16 = mybir.dt.float16
    i16 = mybir.dt.int16

    nc = bacc.Bacc("TRN2", target_bir_lowering=False, debug=False,
                   num_swdge_queues=NQ)

    xs = nc.dram_tensor("xs", [N_VARS, BSLICE], f32, kind="ExternalInput")
    idxa = nc.dram_tensor("idxa", [128, NIDX // 16], i16, kind="ExternalInput")
    idxb = nc.dram_tensor("idxb", [128, NIDX // 16], i16, kind="ExternalInput")
    out = nc.dram_tensor("out", [NOUT, BSLICE], f32, kind="ExternalOutput")
    enc = nc.dram_tensor("enc", [TABLE_ROWS, BSLICE], f16)  # internal scratch

    with TileContext(nc) as tc:
        with tc.tile_pool(name="setup", bufs=1) as sp:
            # load the gather ucode library up front so the pool-config swap
            # doesn't serialize behind the enc build
            nc.gpsimd.load_library(library_config.mlp)

            ia = sp.tile([128, NIDX // 16], i16, tag="ia")
            ib = sp.tile([128, NIDX // 16], i16, tag="ib")

            # ---- build enc table in DRAM (f16) ----
            # quarter-pipelined: loads stream on the sync HWDGE queue,
            # converted f16 quarters write out on the scalar HWDGE queue,
            # so the last enc byte lands ~(last-load + one convert + one
            # write) instead of load+convert+write fully serialized.
            with tc.tile_pool(name="encb", bufs=1) as eb:
                xsr = xs.rearrange("(t p) f -> p t f", p=128)
                encr = enc[0:N_VARS, :].rearrange("(t p) f -> p t f", p=128)
                omxr = enc[N_VARS:2 * N_VARS, :].rearrange(
                    "(t p) f -> p t f", p=128)
                cst = eb.tile([1, 2, BSLICE], f16, tag="cst")
                nc.vector.memset(cst[:, 0, :], 0.0)
                nc.vector.memset(cst[:, 1, :], 1.0)
                nc.scalar.dma_start(
                    out=enc[2 * N_VARS:2 * N_VARS + 2, :]
                        .rearrange("(o r) f -> o r f", o=1),
                    in_=cst[:, :, :])
                for h in range(4):
                    sl = slice(4 * h, 4 * (h + 1))
                    xt = eb.tile([128, 4, BSLICE], f32, tag=f"xt{h}")
                    nc.sync.dma_start(out=xt[:, :, :], in_=xsr[:, sl, :])
                    omx16 = eb.tile([128, 4, BSLICE], f16, tag=f"omx{h}")
                    # 1 - x  ==  (x * -1) + 1, converted to f16
                    nc.vector.tensor_scalar(
                        out=omx16[:, :, :], in0=xt[:, :, :],
                        scalar1=-1.0, scalar2=1.0,
                        op0=mybir.AluOpType.mult, op1=mybir.AluOpType.add)
                    xt16 = eb.tile([128, 4, BSLICE], f16, tag=f"x16{h}")
                    nc.scalar.copy(xt16[:, :, :], xt[:, :, :])
                    nc.scalar.dma_start(out=encr[:, sl, :], in_=xt16[:, :, :])
                    nc.scalar.dma_start(out=omxr[:, sl, :], in_=omx16[:, :, :])

                # index streams ride the sync queue behind the x loads;
                # gen only needs them ~15us later
                nc.sync.dma_start(out=ia[:, :], in_=idxa[:, :])
                nc.sync.dma_start(out=ib[:, :], in_=idxb[:, :])

            # ---- main chunk loop ----
            # dma_gather is limited to 1024 indices per call on HW, so each
            # chunk's stream is gathered in GSUB sub-calls of GI indices,
            # round-robin over the 4 SWDGE queues.
            GI = 1024
            GSUB = CH_IDX // GI
            gcols = GI // 16       # idx columns per sub-gather = 64
            qctr = 0
            with tc.tile_pool(name="gather", bufs=8) as gp, \
                 tc.tile_pool(name="mid", bufs=4) as mp:
                for c in range(NCHUNK):
                    ga = gp.tile([128, 8 * CBLK, BSLICE], f16, tag="ga")
                    gb = gp.tile([128, 8 * CBLK, BSLICE], f16, tag="gb")
                    for k in range(GSUB):
                        col0 = c * (CH_IDX // 16) + k * gcols
                        blk0 = k * (GI // 128)
                        # keep each chunk's A/B on one queue PAIR so the
                        # chunk completes as soon as its pair drains
                        qa = (2 * c) % NQ
                        nc.gpsimd.dma_gather(
                            out_ap=ga[:, blk0:blk0 + GI // 128, :],
                            in_ap=enc[:, :],
                            idxs_ap=ia[:, col0:col0 + gcols],
                            num_idxs=GI, num_idxs_reg=GI,
                            elem_size=BSLICE,
                            queue_num=qa)
                        nc.gpsimd.dma_gather(
                            out_ap=gb[:, blk0:blk0 + GI // 128, :],
                            in_ap=enc[:, :],
                            idxs_ap=ib[:, col0:col0 + gcols],
                            num_idxs=GI, num_idxs_reg=GI,
                            elem_size=BSLICE,
                            queue_num=qa + 1)

                    h0 = mp.tile([128, 8 * CBLK, BSLICE], f16, tag="h0")
                    nc.vector.tensor_mul(h0[:, :, :], ga[:, :, :], gb[:, :, :])
                    h1 = mp.tile([128, 4 * CBLK, BSLICE], f16, tag="h1")
                    nc.vector.tensor_add(
                        h1[:, :, :],
                        h0[:, 0:8 * CBLK:2, :], h0[:, 1:8 * CBLK:2, :])
                    h2 = mp.tile([128, 2 * CBLK, BSLICE], f16, tag="h2")
                    nc.vector.tensor_mul(
                        h2[:, :, :],
                        h1[:, 0:4 * CBLK:2, :], h1[:, 1:4 * CBLK:2, :])
                    h3 = mp.tile([128, CBLK, BSLICE], f32, tag="h3")
                    nc.vector.tensor_add(
                        h3[:, :, :],
                        h2[:, 0:2 * CBLK:2, :], h2[:, 1:2 * CBLK:2, :])

                    nc.sync.dma_start(
                        out=out[c * CBLK * 128:(c + 1) * CBLK * 128, :]
                            .rearrange("(k p) f -> p k f", p=128),
                        in_=h3[:, :, :])

    nc.compile()
    return nc


def _get_program():
    if "nc" not in _CACHED:
        _CACHED["nc"] = _build_program()
    return _CACHED["nc"]


# ----------------------------------------------------------------------------
# public entry point
# ----------------------------------------------------------------------------

def kernel(x, idx0, idx1, idx2, idx3, _trace=False, _trace_kwargs=None):
    from concourse.bass_utils import run_bass_kernel_spmd

    x = np.ascontiguousarray(np.asarray(x, dtype=np.float32))
    sA, sB = _compose_indices(
        np.asarray(idx0), np.asarray(idx1), np.asarray(idx2), np.asarray(idx3))
    wa = [_wrap_idx(_shard_perm(sA, g)) for g in range(GSHARD)]
    wb = [_wrap_idx(_shard_perm(sB, g)) for g in range(GSHARD)]

    nc = _get_program()
    in_maps = []
    for c in range(NCORES):
        g, b = c % GSHARD, c // GSHARD
        xsl = np.ascontiguousarray(x[:, b * BSLICE:(b + 1) * BSLICE])
        in_maps.append({"xs": xsl, "idxa": wa[g], "idxb": wb[g]})

    kwargs = {}
    if _trace:
        kwargs["trace"] = True
        if _trace_kwargs:
            kwargs.update(_trace_kwargs)
    res = run_bass_kernel_spmd(nc, in_maps, core_ids=list(range(NCORES)), **kwargs)
    full = np.empty((NOUT_FULL, BATCH), dtype=np.float32)
    for c in range(NCORES):
        g, b = c % GSHARD, c // GSHARD
        full[g * NOUT:(g + 1) * NOUT, b * BSLICE:(b + 1) * BSLICE] = \
            res.results[c]["out"]
    if _trace:
        kernel.last_exec_time_ns = res.exec_time_ns
        kernel.last_profile = res.profile_json
    return full
